# revision 2
# baseline (speedup 1.0000x reference)
"""Trainium2 Bass kernel for nn_CrossEntropyLossWeight3 — v4 (PE row sums).

Per row b of predict/target [B,16]:
  probs = softmax(predict[b]); pre = argmax(predict[b]); tar = argmax(target[b])
  loss_b = (pre!=tar) * penalty[tar,pre] * probs[pre];  out = mean_b loss_b

On-device identities (per core, data-parallel batch shard):
  probs[pre] = max(exp(x)) / sum(exp(x))
  penalty via per-class counts c embedded (9-bit payload) in the low mantissa
  bits during a fused embed+segmented-max custom DVE op (EMBMAX).

v4 layout of engine work (the v2 baseline was DVE+GPSIMD bound with a
shared-SBUF-port lock serializing the two):
  ACT   : exp (in place, f32) + PSUM->SBUF fp16 ferries
  DVE   : EMBMAX(pred-exp), EMBMAX(targ) + fused formula customs
  PE    : segmented row sums:  transpose e-blocks -> PSUM, ferry to SBUF
          fp16, then mask-matmul (contracts the 16 class lanes) -> s in PSUM
  GPSIMD: idle (avoids the DVE-custom <-> GPSIMD port lock entirely)
Tiles taper at the end ([512,512,512,256,128,64,32,32] rows/partition) so the
pipeline drain is short.
"""

import sys

sys.path.insert(0, "/opt/trn_rl_repo")

import numpy as np

import concourse.bass as bass
import concourse.bacc as bacc
import concourse.tile as tile
from concourse import mybir
from concourse.masks import make_identity
from concourse.bass_utils import run_bass_kernel_spmd

B, W = 2097152, 16
NCORES = 8
BS = B // NCORES
P = 128
RPT = BS // P                 # rows per partition per core (2048)
RS = [256, 256, 256, 256, 256, 256, 256, 128, 64, 32, 32]
BATCHES = [(0, 4), (4, 8), (8, 11)]             # 1024 / 896 / 128 rows
RMAX = max(RS)
FMAX = RMAX * W
assert sum(RS) == RPT

LABELS_NUM_COUNT = [500000, 120000, 80000, 45000, 30000, 250000, 15000, 9000,
                    60000, 7000, 180000, 22000, 11000, 95000, 5000, 40000]

f32 = mybir.dt.float32
fp16 = mybir.dt.float16
u32 = mybir.dt.uint32
AX = mybir.AxisListType
OP = mybir.AluOpType
ACT = mybir.ActivationFunctionType

PAYLOAD_BITS = 9
PAYLOAD_MASK = (1 << PAYLOAD_BITS) - 1


def _register_custom_ops():
    """EMBMAX_SEG_ANT: fused (clear low 9 mantissa bits, OR payload) +
    segmented max over [P, S, 16] with stride-0 out -> [P, S].
    SMN2_ANT: from embedded maxes: (u!=v) ? max(u,v) : 0, u/v = payloads.
    DEN2_ANT: u + v."""
    import numpy as np_

    from concourse.dve_spec import (
        Spec, Src0, Src1, C0, C1, Bin, AluOp, lower, Zero,
    )
    from concourse.dve_ops import (
        DveOp, OPS, CUSTOM_DVE_SPECS, _SUB_OPCODE_FOR_NAME,
        _CUSTOM_DVE_ROW_BASE, _COMPILE_CACHE,
    )
    from concourse.dve_uop import DveOpSpec
    import concourse.dve_spec as ds

    def reg(name, spec, rd1):
        for o in OPS:
            if o.name == name:
                return o
        shas = {}
        opcode = _CUSTOM_DVE_ROW_BASE + len(OPS)
        for ver in ("v3", "v4"):
            uops = lower(spec, ver=ver)
            s = DveOpSpec(name=name, opcode=opcode, uops=uops, rd1_en=rd1)
            shas[ver] = s.sha(ver)
            _COMPILE_CACHE[(name, ver)] = s
        op = DveOp(name, spec, subdim=False, uops_sha=shas)
        _SUB_OPCODE_FOR_NAME[name] = opcode
        OPS.append(op)
        CUSTOM_DVE_SPECS[name] = spec
        return op

    embed_expr = Bin(
        AluOp.BITWISE_OR,
        Bin(AluOp.BITWISE_XOR, Bin(AluOp.BITWISE_OR, Src0, C0), C0),
        Src1,
    )

    def _ref_embmax(in0, in1, s0, s1, imm2):
        emb = (
            ((in0.view(np_.uint32) | PAYLOAD_MASK) ^ PAYLOAD_MASK)
            | in1.view(np_.uint32)
        ).view(np_.float32)
        return np_.maximum.accumulate(emb, axis=-1)

    def reg_embmax():
        name = "EMBMAX_SEG_ANT"
        for o in OPS:
            if o.name == name:
                return o
        seg = ds.Scan(op=AluOp.MAX, expr=embed_expr, init=None, _subdim_step=Zero)
        spec = Spec(body=seg, reference=_ref_embmax)
        orig_so, orig_nas = ds._scan_overrides, ds._node_as_stage

        def patched_so(scans, node_stage):
            seed, step = {}, {}
            for scan in scans:
                d = node_stage[scan]
                init = (
                    scan.init if scan.init is not None
                    else ds._ACCUM_IDENTITY[scan.op]
                )
                seed[d] = orig_nas(init)
                if scan._subdim_step is not None:
                    step[d] = ds._Stage(AluOp.BYPASS, scan.expr)
            return seed, step

        def patched_nas(e):
            if isinstance(e, ds.Scan) and e._subdim_step is not None:
                return ds._Stage(e.op, ds.AluInp.CURR_ALU_OUT, e.expr)
            return orig_nas(e)

        uops_by_ver, shas = {}, {}
        ds._scan_overrides, ds._node_as_stage = patched_so, patched_nas
        try:
            for ver in ("v3", "v4"):
                uops_by_ver[ver] = lower(spec, ver=ver)
        finally:
            ds._scan_overrides, ds._node_as_stage = orig_so, orig_nas
        opcode = _CUSTOM_DVE_ROW_BASE + len(OPS)
        for ver in ("v3", "v4"):
            s = DveOpSpec(name=name, opcode=opcode, uops=uops_by_ver[ver],
                          rd1_en=True)
            shas[ver] = s.sha(ver)
            _COMPILE_CACHE[(name, ver)] = s
        op = DveOp(name, spec, subdim=True, uops_sha=shas)
        _SUB_OPCODE_FOR_NAME[name] = opcode
        OPS.append(op)
        CUSTOM_DVE_SPECS[name] = spec
        return op

    ut = Bin(AluOp.BITWISE_OR, Bin(AluOp.BITWISE_AND, Src0, C0), C1)
    vt = Bin(AluOp.BITWISE_OR, Bin(AluOp.BITWISE_AND, Src1, C0), C1)

    def _ref_smn2(in0, in1, s0, s1, imm2):
        ub = (in0.view(np_.uint32) & PAYLOAD_MASK).astype(np_.float32)
        vb = (in1.view(np_.uint32) & PAYLOAD_MASK).astype(np_.float32)
        return np_.where(ub != vb, np_.maximum(ub, vb), 0.0).astype(np_.float32)

    def _ref_den2(in0, in1, s0, s1, imm2):
        ub = (in0.view(np_.uint32) & PAYLOAD_MASK).astype(np_.float32)
        vb = (in1.view(np_.uint32) & PAYLOAD_MASK).astype(np_.float32)
        return (ub + vb).astype(np_.float32)

    embed = reg_embmax()
    smn2 = reg(
        "SMN2_ANT",
        Spec(
            body=Bin(
                AluOp.MULTIPLY,
                Bin(AluOp.SUBTRACT, Bin(AluOp.MAX, ut, vt), C1),
                Bin(AluOp.IS_NE, ut, vt),
            ),
            reference=_ref_smn2,
        ),
        rd1=True,
    )
    den2 = reg(
        "DEN2_ANT",
        Spec(
            body=Bin(
                AluOp.SUBTRACT,
                Bin(AluOp.ADD, Bin(AluOp.SUBTRACT, ut, C1), vt), C1,
            ),
            reference=_ref_den2,
        ),
        rd1=True,
    )
    return embed, smn2, den2


def _build_program():
    nc = bacc.Bacc("TRN2", target_bir_lowering=False, debug=False)
    pred = nc.dram_tensor("predict", [BS, W], f32, kind="ExternalInput")
    targ = nc.dram_tensor("target", [BS, W], f32, kind="ExternalInput")
    pay = nc.dram_tensor("payload", [P, W], u32, kind="ExternalInput")
    mmm_d = nc.dram_tensor("mmmask", [P, 8], f32, kind="ExternalInput")
    out = nc.dram_tensor("out", [P, 1], f32, kind="ExternalOutput")

    embed_op, smn2_op, den2_op = _register_custom_ops()

    with tile.TileContext(nc) as tc:
        with (
            tc.tile_pool(name="ioxp", bufs=4) as ioxp,
            tc.tile_pool(name="ioxt", bufs=3) as ioxt,
            tc.tile_pool(name="fer", bufs=2) as fer,
            tc.tile_pool(name="stats", bufs=2) as stats,
            tc.tile_pool(name="scr", bufs=1) as scr,
            tc.tile_pool(name="const", bufs=1) as cpool,
            tc.tile_pool(name="pst", bufs=2, space="PSUM") as pst,
            tc.tile_pool(name="pss", bufs=2, space="PSUM") as pss,
        ):
            pay_t = cpool.tile([P, W], u32, tag="pay")
            nc.gpsimd.dma_start(out=pay_t[:, :], in_=pay[:, :])
            mask_t = cpool.tile([P, 1], u32, tag="mask")
            nc.vector.memset(mask_t[:, :], PAYLOAD_MASK)
            mask_ap = mask_t[:, :1].bitcast(f32)
            mm_f32 = cpool.tile([P, 8], f32, tag="mmf")
            nc.sync.dma_start(out=mm_f32[:, :], in_=mmm_d[:, :])
            mm16 = cpool.tile([P, 8], fp16, tag="mm16")
            nc.scalar.copy(mm16[:, :], mm_f32[:, :])
            ident = cpool.tile([P, P], f32, tag="ident")
            make_identity(nc, ident[:, :])

            accs = []
            for i in range(len(BATCHES)):
                acc_i = cpool.tile([P, 1], f32, tag=f"acc{i}", name=f"acc{i}")
                accs.append(acc_i)

            ro = 0
            for bi, (t0, t1) in enumerate(BATCHES):
                bcols = sum(RS[t0:t1])
                me2 = stats.tile([P, 1024], f32, tag="me2")
                mt2 = stats.tile([P, 1024], f32, tag="mt2")
                s2 = pss.tile([P, 1024], f32, tag="s2")
                bc = 0
                for t in range(t0, t1):
                    Rt = RS[t]
                    Ft = Rt * W
                    rows = slice(ro * P, (ro + Rt) * P)
                    pv = pred[rows, :].rearrange("(p r) w -> p (r w)", p=P)
                    tv = targ[rows, :].rearrange("(p r) w -> p (r w)", p=P)

                    xp = ioxp.tile([P, FMAX], f32, tag="xp")
                    nc.sync.dma_start(out=xp[:, :Ft], in_=pv)
                    xt = ioxt.tile([P, FMAX], f32, tag="xt")
                    nc.sync.dma_start(out=xt[:, :Ft], in_=tv)

                    # e = exp(predict) in place (ScalarE)
                    e = xp
                    nc.scalar.activation(e[:, :Ft], xp[:, :Ft], ACT.Exp)
                    e3 = e[:, :Ft].rearrange("p (r w) -> p r w", w=W)
                    pay_b = (pay_t[:, :].unsqueeze(1)
                             .broadcast_to([P, Rt, W]).bitcast(f32))

                    # fused embed+segmented-max (DVE custom), both sides
                    nc.vector._custom_dve(
                        embed_op,
                        out=me2[:, bc:bc+Rt].unsqueeze(2)
                            .broadcast_to([P, Rt, W]),
                        in0=e3, in1=pay_b, s0=mask_ap,
                    )
                    xt3 = xt[:, :Ft].rearrange("p (r w) -> p r w", w=W)
                    nc.vector._custom_dve(
                        embed_op,
                        out=mt2[:, bc:bc+Rt].unsqueeze(2)
                            .broadcast_to([P, Rt, W]),
                        in0=xt3, in1=pay_b, s0=mask_ap,
                    )

                    # PE segmented row sums of e, 8-block chunks
                    nblk = Ft // 128
                    for c0 in range(0, nblk, 8):
                        cb = min(8, nblk - c0)
                        eT = pst.tile([P, 1024], f32, tag="eT")
                        for j in range(cb):
                            col = (c0 + j) * 128
                            nc.tensor.transpose(
                                eT[:, j*128:(j+1)*128],
                                e[:, col:col+128], ident[:, :])
                        eTs = fer.tile([P, 1024], fp16, tag="eTs")
                        nc.scalar.copy(eTs[:, :cb*128], eT[:, :cb*128])
                        for j in range(cb):
                            oc = bc + (c0 + j) * 8
                            nc.tensor.matmul(
                                out=s2[:, oc:oc+8],
                                lhsT=eTs[:, j*128:(j+1)*128],
                                rhs=mm16[:, :],
                            )
                    bc += Rt
                    ro += Rt

                # formula over the batch [P, bcols]
                sl = slice(0, bcols)
                smn = scr.tile([P, 1024], f32, tag="smn")
                nc.vector._custom_dve(
                    smn2_op, out=smn[:, sl], in0=me2[:, sl], in1=mt2[:, sl],
                    s0=mask_ap, s1=8388608.0)
                den = scr.tile([P, 1024], f32, tag="den")
                nc.vector._custom_dve(
                    den2_op, out=den[:, sl], in0=me2[:, sl], in1=mt2[:, sl],
                    s0=mask_ap, s1=8388608.0)
                # num = smn * me2 (me2 payload bits: <=2^-14 rel perturbation)
                nc.vector.tensor_tensor(smn[:, sl], smn[:, sl], me2[:, sl],
                                        op=OP.mult)
                nc.vector.tensor_tensor(den[:, sl], den[:, sl], s2[:, sl],
                                        op=OP.mult)
                rec = scr.tile([P, 1024], f32, tag="rec")
                nc.vector.reciprocal_approx_fast(out=rec[:, sl], in_=den[:, sl])
                nc.vector.tensor_tensor(smn[:, sl], smn[:, sl], rec[:, sl],
                                        op=OP.mult)
                nc.vector.reduce_sum(accs[bi][:, :], smn[:, sl], axis=AX.X)
                if bi > 0:
                    nc.vector.tensor_tensor(
                        accs[bi][:, :], accs[bi][:, :], accs[bi - 1][:, :],
                        op=OP.add)

            nc.sync.dma_start(out=out[:, :], in_=accs[-1][:, :])
    nc.compile()
    return nc


_CACHE = {}


def _run(predict, target, trace=False):
    if "nc" not in _CACHE:
        _CACHE["nc"] = _build_program()
    nc = _CACHE["nc"]

    predict = np.ascontiguousarray(np.asarray(predict, dtype=np.float32))
    target = np.ascontiguousarray(np.asarray(target, dtype=np.float32))
    payload = np.broadcast_to(
        (np.asarray(LABELS_NUM_COUNT, dtype=np.uint32) // 1000)[None, :], (P, W)
    ).copy()
    q = np.arange(P)
    mmm = (q[:, None] // 16 == np.arange(8)[None, :]).astype(np.float32)

    in_maps = []
    for i in range(NCORES):
        in_maps.append(
            {
                "predict": predict[i * BS: (i + 1) * BS],
                "target": target[i * BS: (i + 1) * BS],
                "payload": payload,
                "mmmask": mmm,
            }
        )
    res = run_bass_kernel_spmd(nc, in_maps, core_ids=list(range(NCORES)),
                               trace=trace)
    total = np.float64(0.0)
    for r in res.results:
        total += np.float64(r["out"].astype(np.float64).sum())
    value = np.float32(total / B)
    return np.asarray(value, dtype=np.float32), res


def kernel(predict, target, penalty_matrix=None):
    value, _ = _run(predict, target, trace=False)
    return value
